# revision 1
# baseline (speedup 1.0000x reference)
"""Trainium2 Bass kernel for LiftSplatShoot voxel pooling (segment_reduce).

kernel(**inputs) takes the FULL inputs and returns the FULL output
(B, NZ*C, NY, NX) float32.

Strategy (8 NeuronCores = 4 batches x 2 BEV-grid halves, fully disjoint):
  host: replicate the reference geometry with eager jnp ops (bit-identical
        voxel assignment), sort each core's points by dense output row, pad
        every voxel run to a multiple of 16 ("groups"), pack voxel-atomic
        chunks of <=128 groups, and pre-gather x into the device layout.
  device (SPMD, per chunk):
        DMA x tile [128 groups, 16*64] -> DVE tree-folds the 16 members ->
        group sums [128,64]; onehot(group_slot)=is_equal(slot, iota) ->
        PE matmul segment-reduce into PSUM [128 slots, 64] -> staged in SBUF;
        every S chunks one dma_scatter_add accumulates the staged slot sums
        into their dense rows (outputs are zero-initialized by the runtime;
        unused slots add +0.0 into a known-empty row).
  host: concatenate the 8 disjoint dense sub-grids and transpose to
        (B, C, NY, NX).
"""
import numpy as np

# ---- static problem config (hardcoded per contest rules) ----
B, N, C, D = 4, 4, 64, 41
OGH, OGW, DS = 256, 704, 16
FH, FW = OGH // DS, OGW // DS  # 16, 44
XB = (-51.2, 51.2, 0.4)
YB = (-51.2, 51.2, 0.4)
ZB = (-10.0, 10.0, 20.0)
NX, NY, NZ = 256, 256, 1
NP = B * N * D * FH * FW

CH = 64     # channels per point row
G = 16      # members per group
VC = NZ * NY * NX // 2  # dense rows per core (half a batch grid) = 32768
NGC = 30    # chunks per core (max over cores is 30; others padded)
S = 6       # chunks per scatter instruction
NSCAT = NGC // S
TOK = S * 128

_CACHE = {}


def _geometry_rows(rots, trans, intrins, post_rots, post_trans):
    """Replicate reference geometry exactly (same eager jnp ops) and return
    the global flat voxel index per point and the kept mask (numpy).

    Runs on the jax CPU backend: the axon/neuron backend cannot lower
    jnp.linalg.inv (triangular-solve unsupported), and the grading reference
    must therefore run on CPU as well — matching its numerics bit-for-bit.
    """
    import jax
    import jax.numpy as jnp
    cpu = jax.local_devices(backend="cpu")[0]
    with jax.default_device(cpu):
        return _geometry_rows_impl(jnp, rots, trans, intrins, post_rots,
                                   post_trans)


def _geometry_rows_impl(jnp, rots, trans, intrins, post_rots, post_trans):
    rots = jnp.asarray(rots)
    trans = jnp.asarray(trans)
    intrins = jnp.asarray(intrins)
    post_rots = jnp.asarray(post_rots)
    post_trans = jnp.asarray(post_trans)

    dx = jnp.array([XB[2], YB[2], ZB[2]], jnp.float32)
    bx = jnp.array([XB[0] + XB[2] / 2.0, YB[0] + YB[2] / 2.0,
                    ZB[0] + ZB[2] / 2.0], jnp.float32)
    ds = (2.0 + jnp.arange(D, dtype=jnp.float32)).reshape(D, 1, 1) \
        * jnp.ones((1, FH, FW), jnp.float32)
    xs = jnp.linspace(0.0, OGW - 1, FW, dtype=jnp.float32).reshape(1, 1, FW) \
        * jnp.ones((D, FH, 1), jnp.float32)
    ys = jnp.linspace(0.0, OGH - 1, FH, dtype=jnp.float32).reshape(1, FH, 1) \
        * jnp.ones((D, 1, FW), jnp.float32)
    frustum = jnp.stack([xs, ys, ds], -1)

    pts = frustum[None, None] - post_trans[:, :, None, None, None, :]
    pts = jnp.einsum('bnij,bndhwj->bndhwi', jnp.linalg.inv(post_rots), pts)
    pts = jnp.concatenate([pts[..., :2] * pts[..., 2:3], pts[..., 2:3]], -1)
    combine = rots @ jnp.linalg.inv(intrins)
    geom = jnp.einsum('bnij,bndhwj->bndhwi', combine, pts) \
        + trans[:, :, None, None, None, :]

    vox = jnp.floor((geom.reshape(NP, 3) - (bx - dx / 2.0)) / dx).astype(jnp.int32)
    vox = np.asarray(vox)
    kept = (vox[:, 0] >= 0) & (vox[:, 0] < NX) & (vox[:, 1] >= 0) \
        & (vox[:, 1] < NY) & (vox[:, 2] >= 0) & (vox[:, 2] < NZ)
    bix = np.repeat(np.arange(B, dtype=np.int64), NP // B)
    flat = ((bix * NZ + vox[:, 2].astype(np.int64)) * NY + vox[:, 1]) * NX + vox[:, 0]
    return flat, kept


def _build_kernel():
    import concourse.bacc as bacc
    import concourse.mybir as mybir
    import concourse.tile as tile
    F32 = mybir.dt.float32
    I16 = mybir.dt.int16

    nc = bacc.Bacc("TRN2", target_bir_lowering=False, debug=False,
                   num_devices=8)
    xd = nc.dram_tensor("xd", [NGC, 128, G * CH], F32, kind="ExternalInput")
    gslots = nc.dram_tensor("gslots", [NGC, 128, 1], F32, kind="ExternalInput")
    idxs = nc.dram_tensor("idxs", [NSCAT, 128, TOK // 16], I16,
                          kind="ExternalInput")
    out = nc.dram_tensor("out", [VC, CH], F32, kind="ExternalOutput")
    with tile.TileContext(nc) as tc:
        with (
            tc.tile_pool(name="const", bufs=1) as cp,
            tc.tile_pool(name="xp", bufs=4) as xpool,
            tc.tile_pool(name="ohp", bufs=3) as ohpool,
            tc.tile_pool(name="ps2", bufs=3, space="PSUM") as ps2pool,
            tc.tile_pool(name="stg", bufs=2) as stgpool,
            tc.tile_pool(name="idxp", bufs=2) as idxpool,
            tc.tile_pool(name="gsum", bufs=4) as gsumpool,
        ):
            iota_t = cp.tile([128, 128], F32)
            nc.gpsimd.iota(iota_t[:], pattern=[[1, 128]], base=0,
                           channel_multiplier=0,
                           allow_small_or_imprecise_dtypes=True)
            gs_all = cp.tile([128, NGC], F32)
            nc.sync.dma_start(out=gs_all[:],
                              in_=gslots[:].rearrange("k p one -> p (k one)"))
            for g in range(NSCAT):
                idx_t = idxpool.tile([128, TOK // 16], I16)
                nc.sync.dma_start(out=idx_t[:], in_=idxs[g])
                stage_t = stgpool.tile([128, S, CH], F32)
                for kl in range(S):
                    k = g * S + kl
                    x_t = xpool.tile([128, G * CH], F32)
                    nc.sync.dma_start(out=x_t[:], in_=xd[k])
                    v = x_t
                    nc.vector.tensor_add(out=v[:, 0:8 * CH], in0=v[:, 0:8 * CH],
                                         in1=v[:, 8 * CH:16 * CH])
                    nc.vector.tensor_add(out=v[:, 0:4 * CH], in0=v[:, 0:4 * CH],
                                         in1=v[:, 4 * CH:8 * CH])
                    nc.vector.tensor_add(out=v[:, 0:2 * CH], in0=v[:, 0:2 * CH],
                                         in1=v[:, 2 * CH:4 * CH])
                    gsum_t = gsumpool.tile([128, CH], F32)
                    nc.vector.tensor_add(out=gsum_t[:], in0=v[:, 0:CH],
                                         in1=v[:, CH:2 * CH])
                    oh_t = ohpool.tile([128, 128], F32)
                    nc.vector.tensor_tensor(
                        out=oh_t[:],
                        in0=gs_all[:, k:k + 1].to_broadcast([128, 128]),
                        in1=iota_t[:], op=mybir.AluOpType.is_equal)
                    ps2_t = ps2pool.tile([128, CH], F32)
                    nc.tensor.matmul(out=ps2_t[:], lhsT=oh_t[:], rhs=gsum_t[:],
                                     start=True, stop=True)
                    nc.vector.tensor_copy(out=stage_t[:, kl, :], in_=ps2_t[:])
                nc.gpsimd.dma_scatter_add(out[:], stage_t[:], idx_t[:], TOK,
                                          TOK, CH)
    nc.finalize()
    return nc


def _plan_core(rows, order):
    """rows ascending (local dense rows in [0, VC)); order: matching global
    point indices."""
    uniq, counts = np.unique(rows, return_counts=True)
    used = set(uniq.tolist())
    dump = next(r for r in range(VC) if r not in used)

    chunks = []
    cur, cur_groups = [], 0
    pos = 0
    for r, c in zip(uniq.tolist(), counts.tolist()):
        ng = -(-c // G)
        assert ng <= 128, f"voxel run {c} needs {ng} groups"
        if cur_groups + ng > 128:
            chunks.append(cur)
            cur, cur_groups = [], 0
        cur.append((r, pos, c, ng))
        cur_groups += ng
        pos += c
    if cur:
        chunks.append(cur)
    nck = len(chunks)
    assert nck <= NGC, f"core needs {nck} chunks > NGC={NGC}"

    gslot = np.zeros((NGC, 128), np.float32)
    chunk_rows = np.full((NGC, 128), dump, np.int32)
    gather = np.full((NGC, 128, G), -1, np.int64)
    for k, ch in enumerate(chunks):
        gi = 0
        for si, (r, start, cnt, ng) in enumerate(ch):
            chunk_rows[k, si] = r
            for j in range(ng):
                lo = start + j * G
                hi = start + min((j + 1) * G, cnt)
                gslot[k, gi] = si
                gather[k, gi, :hi - lo] = order[lo:hi]
                gi += 1
        assert gi <= 128
    return gslot, chunk_rows, gather


def _core_inputs(gslot, chunk_rows, gather, xf_ext):
    gidx = gather.copy()
    gidx[gidx < 0] = xf_ext.shape[0] - 1
    xd = xf_ext[gidx.reshape(-1)].reshape(NGC, 128, G * CH)

    idx_tok = chunk_rows.reshape(NSCAT, TOK)
    idxs16 = np.zeros((NSCAT, 16, TOK // 16), np.int16)
    t = np.arange(TOK)
    idxs16[:, t % 16, t // 16] = idx_tok.astype(np.int16)
    idxs = np.tile(idxs16, (1, 8, 1))
    return dict(xd=np.ascontiguousarray(xd),
                gslots=np.ascontiguousarray(gslot[:, :, None]),
                idxs=np.ascontiguousarray(idxs))


def kernel(x, rots, trans, intrins, post_rots, post_trans):
    from concourse.bass_utils import run_bass_kernel_spmd

    x = np.asarray(x, dtype=np.float32)
    flat, kept = _geometry_rows(rots, trans, intrins, post_rots, post_trans)

    xf = x.reshape(NP, CH)
    xf_ext = np.concatenate([xf, np.zeros((1, CH), np.float32)], axis=0)

    in_maps = []
    for core in range(8):
        b, half = core // 2, core % 2
        lo = b * (NZ * NY * NX) + half * VC
        m = kept & (flat >= lo) & (flat < lo + VC)
        local = (flat[m] - lo).astype(np.int64)
        order = np.nonzero(m)[0]
        srt = np.argsort(local, kind="stable")
        gslot, chunk_rows, gather = _plan_core(local[srt], order[srt])
        in_maps.append(_core_inputs(gslot, chunk_rows, gather, xf_ext))

    if "nc" not in _CACHE:
        _CACHE["nc"] = _build_kernel()
    nc = _CACHE["nc"]

    res = run_bass_kernel_spmd(nc, in_maps, core_ids=list(range(8)))

    final = np.empty((B, NZ * C, NY, NX), np.float32)
    for core in range(8):
        b, half = core // 2, core % 2
        o = np.asarray(res.results[core]["out"])  # (VC, CH)
        o = o.reshape(NY // 2, NX, CH).transpose(2, 0, 1)  # (CH, 128, 256)
        final[b, :, half * (NY // 2):(half + 1) * (NY // 2), :] = o
    return final



# revision 33
# speedup vs baseline: 2.1366x; 2.1366x over previous
"""Trainium2 Bass kernel for LiftSplatShoot voxel pooling (segment_reduce).

kernel(**inputs) takes the FULL inputs and returns the FULL output
(B, NZ*C, NY, NX) float32.

Strategy (8 NeuronCores, globally balanced, fully disjoint):
  host: replicate the reference geometry with eager jnp ops on CPU
        (bit-identical voxel assignment), sort all kept points by dense
        output row, round every voxel run up to whole groups of 16
        members, bin-pack voxel runs (FFD) into chunks of <=128 groups,
        split the chunk list evenly across the 8 cores, and pre-gather x
        (cast to bf16) into the device layout [chunk, group, member, ch].
  device (SPMD, per chunk):
        DMA x tile [128 groups, 16mem x 64ch] bf16; the PE does the whole
        reduction: 16 accumulating matmuls (one per member column block)
        with a bf16 onehot(group->slot) lhsT generated on the Pool engine
        -> PSUM [128 slots, 64] f32 voxel sums; the Act engine copies
        PSUM to bf16 SBUF staging; one DMA per scatter-group of chunks
        writes the staged slot sums to DRAM (compact layout, no scatter).
        The last chunk holds 128 reserved single-group voxels and is
        folded by the otherwise-idle DVE straight into the final staging
        tile, keeping the drain tail short.
  host: scatter the compact per-(chunk,slot) voxel sums into the dense
        BEV grid (pure indexing, each voxel written exactly once), then
        transpose to (B, NZ*C, NY, NX) float32.
"""
import numpy as np
import ml_dtypes

BF = ml_dtypes.bfloat16

# ---- static problem config (hardcoded per contest rules) ----
B, N, C, D = 4, 4, 64, 41
OGH, OGW, DS = 256, 704, 16
FH, FW = OGH // DS, OGW // DS  # 16, 44
XB = (-51.2, 51.2, 0.4)
YB = (-51.2, 51.2, 0.4)
ZB = (-10.0, 10.0, 20.0)
NX, NY, NZ = 256, 256, 1
NP = B * N * D * FH * FW
NROWS = B * NZ * NY * NX

CH = 64     # channels per point row
G = 16      # members per group
NCORES = 8
SMAXG = 7   # max chunks per output staging group
XPC = 2     # chunks per input DMA

_CACHE = {}


def _geometry_rows(rots, trans, intrins, post_rots, post_trans):
    """Replicate reference geometry exactly (same eager jnp ops) and return
    the global flat voxel index per point and the kept mask (numpy).

    Runs on the jax CPU backend: the axon/neuron backend cannot lower
    jnp.linalg.inv (triangular-solve unsupported), and the grading reference
    must therefore run on CPU as well — matching its numerics bit-for-bit.
    """
    import jax
    cpu = jax.local_devices(backend="cpu")[0]
    with jax.default_device(cpu):
        return _geometry_rows_impl(rots, trans, intrins, post_rots, post_trans)


def _geometry_rows_impl(rots, trans, intrins, post_rots, post_trans):
    import jax.numpy as jnp
    rots = jnp.asarray(rots)
    trans = jnp.asarray(trans)
    intrins = jnp.asarray(intrins)
    post_rots = jnp.asarray(post_rots)
    post_trans = jnp.asarray(post_trans)

    dx = jnp.array([XB[2], YB[2], ZB[2]], jnp.float32)
    bx = jnp.array([XB[0] + XB[2] / 2.0, YB[0] + YB[2] / 2.0,
                    ZB[0] + ZB[2] / 2.0], jnp.float32)
    ds = (2.0 + jnp.arange(D, dtype=jnp.float32)).reshape(D, 1, 1) \
        * jnp.ones((1, FH, FW), jnp.float32)
    xs = jnp.linspace(0.0, OGW - 1, FW, dtype=jnp.float32).reshape(1, 1, FW) \
        * jnp.ones((D, FH, 1), jnp.float32)
    ys = jnp.linspace(0.0, OGH - 1, FH, dtype=jnp.float32).reshape(1, FH, 1) \
        * jnp.ones((D, 1, FW), jnp.float32)
    frustum = jnp.stack([xs, ys, ds], -1)

    pts = frustum[None, None] - post_trans[:, :, None, None, None, :]
    pts = jnp.einsum('bnij,bndhwj->bndhwi', jnp.linalg.inv(post_rots), pts)
    pts = jnp.concatenate([pts[..., :2] * pts[..., 2:3], pts[..., 2:3]], -1)
    combine = rots @ jnp.linalg.inv(intrins)
    geom = jnp.einsum('bnij,bndhwj->bndhwi', combine, pts) \
        + trans[:, :, None, None, None, :]

    vox = jnp.floor((geom.reshape(NP, 3) - (bx - dx / 2.0)) / dx).astype(jnp.int32)
    vox = np.asarray(vox)
    kept = (vox[:, 0] >= 0) & (vox[:, 0] < NX) & (vox[:, 1] >= 0) \
        & (vox[:, 1] < NY) & (vox[:, 2] >= 0) & (vox[:, 2] < NZ)
    bix = np.repeat(np.arange(B, dtype=np.int64), NP // B)
    flat = ((bix * NZ + vox[:, 2].astype(np.int64)) * NY + vox[:, 1]) * NX + vox[:, 0]
    return flat, kept


def _plan(flat, kept):
    """Bin-pack voxel runs into chunks of <=128 groups of <=16 members.

    Per core, the LAST chunk is "special": 128 reserved single-group voxels,
    folded by the DVE without the matmul machinery (short drain tail).
    Returns (ngc, gather, gslot, rows, sp_gather, sp_rows) where
      gather [NREG_TOT, 128, G] int64: point index per member (NP = zero row)
      gslot  [NREG_TOT, 128] uint8: output slot per group partition
      rows   [NREG_TOT, 128] int64: global dense row per slot (-1 unused)
      sp_gather [NCORES, 128, G] int64: special-chunk member indices
      sp_rows   [NCORES, 128] int64: special-chunk dense rows (-1 unused)
    NREG_TOT = NCORES * (ngc - 1); core c owns regular chunks
    [c*(ngc-1), (c+1)*(ngc-1)) plus its special chunk.
    """
    idx = np.nonzero(kept)[0]
    rows_k = flat[idx]
    order = np.argsort(rows_k, kind="stable")
    idx = idx[order]
    rows_k = rows_k[order]
    uniq, starts, counts = np.unique(rows_k, return_index=True,
                                     return_counts=True)
    nvox = len(uniq)
    ngroups = (-(-counts // G)).astype(np.int64)
    assert ngroups.max() <= 128, f"voxel needs {ngroups.max()} groups"

    # reserve 128 single-group voxels per core for the special chunks
    single = np.nonzero(ngroups == 1)[0]
    assert len(single) >= NCORES * 128, "not enough single-group voxels"
    reserved = single[:NCORES * 128]
    is_res = np.zeros(nvox, bool)
    is_res[reserved] = True

    total_groups = int(ngroups[~is_res].sum())

    # First-fit-decreasing bin packing of voxel runs into 128-group chunks.
    vorder = np.argsort(-ngroups, kind="stable")
    vorder = vorder[~is_res[vorder]]
    nbins = -(-total_groups // 128)
    nbins = max(nbins, 1)
    while True:
        caps = np.full(nbins, 128, np.int64)
        vbin = np.full(nvox, -1, np.int64)
        ok = True
        for v in vorder:
            g = ngroups[v]
            fits = np.nonzero(caps >= g)[0]
            if len(fits) == 0:
                ok = False
                break
            b = fits[0]
            vbin[v] = b
            caps[b] -= g
        if ok:
            break
        nbins += 1

    nreg = -(-nbins // NCORES)          # regular chunks per core
    ngc = nreg + 1
    ntot = NCORES * nreg

    gather = np.full((ntot, 128, G), NP, np.int64)
    gslot = np.zeros((ntot, 128), np.uint8)
    rows = np.full((ntot, 128), -1, np.int64)

    # assign slots/partitions within each bin in voxel order
    binslot = np.zeros(nbins, np.int64)   # next slot id per bin
    binpart = np.zeros(nbins, np.int64)   # next group partition per bin
    for v in range(nvox):
        b = vbin[v]
        if b < 0:
            continue
        s = binslot[b]
        binslot[b] += 1
        rows[b, s] = uniq[v]
        c = int(counts[v])
        st = int(starts[v])
        ng = int(ngroups[v])
        p0 = binpart[b]
        binpart[b] += ng
        for j in range(ng):
            lo = st + j * G
            hi = st + min((j + 1) * G, c)
            gather[b, p0 + j, :hi - lo] = idx[lo:hi]
            gslot[b, p0 + j] = s
    assert binslot.max() <= 128 and binpart.max() <= 128

    sp_gather = np.full((NCORES, 128, G), NP, np.int64)
    sp_rows = np.full((NCORES, 128), -1, np.int64)
    for c in range(NCORES):
        vs = reserved[c * 128:(c + 1) * 128]
        for s, v in enumerate(vs):
            cnt = int(counts[v])
            st = int(starts[v])
            sp_gather[c, s, :cnt] = idx[st:st + cnt]
            sp_rows[c, s] = uniq[v]
    return ngc, gather, gslot, rows, sp_gather, sp_rows


def _split_groups(nreg):
    nscat = -(-nreg // SMAXG)
    base = nreg // nscat
    rem = nreg - base * nscat
    return [base + (1 if i < rem else 0) for i in range(nscat)]


def _build_kernel(ngc):
    import concourse.bacc as bacc
    import concourse.mybir as mybir
    import concourse.tile as tile
    F32 = mybir.dt.float32
    BF16 = mybir.dt.bfloat16

    nreg = ngc - 1
    groups = _split_groups(nreg)
    nscat = len(groups)
    smax = max(groups)
    w3 = (groups[-1] + 1) * CH          # last stage incl. special column

    step3 = -(-w3 // 128) * 128         # scatter stride must be 256B-aligned

    nc = bacc.Bacc("TRN2", target_bir_lowering=False, debug=False,
                   num_devices=NCORES)
    xd = nc.dram_tensor("xd", [ngc, 128, G * CH], BF16, kind="ExternalInput")
    gsd = nc.dram_tensor("gsd", [128, nreg], BF16, kind="ExternalInput")
    idxd = nc.dram_tensor("idxd", [128, 8], mybir.dt.int16,
                          kind="ExternalInput")
    outc = nc.dram_tensor("outc", [nscat - 1, 128, smax, CH], BF16,
                          kind="ExternalOutput")
    outl = nc.dram_tensor("outl", [128, step3], BF16, kind="ExternalOutput")

    # the final staging buffer lives in a hand-allocated arena with TWO
    # aliased handles: compute writes stage_l (tracked), while the early
    # SWDGE descriptor prep reads the alias — so its ~1us desc generation is
    # NOT ordered after the last write.  trigger_dma fires the transfer at
    # the end, skipping the ~1.3us HWDGE+DGE latency of a plain DMA; a tiny
    # Pool-engine read of stage_l fences the trigger behind all writes.
    arena = nc.alloc_sbuf_tensor("stage_arena", [128, w3], BF16)
    addr = nc.lookup_mloc(arena).addr
    stage_l = nc.alloc_sbuf_tensor_at("stage_l_w", [128, w3], BF16,
                                      offset=addr)
    stage_lr = nc.alloc_sbuf_tensor_at("stage_l_r", [128, w3], BF16,
                                       offset=addr)

    with tile.TileContext(nc) as tc, \
            nc.allow_low_precision(reason="bf16 voxel pooling"):
        with (
            tc.tile_pool(name="const", bufs=1) as cp,
            tc.tile_pool(name="xp", bufs=6) as xpool,
            tc.tile_pool(name="ohp", bufs=2) as ohpool,
            tc.tile_pool(name="psp", bufs=4, space="PSUM") as pspool,
            tc.tile_pool(name="stg", bufs=nscat) as stgpool,
        ):
            iota_t = cp.tile([128, 128], BF16)
            nc.gpsimd.iota(iota_t[:], pattern=[[1, 128]], base=0,
                           channel_multiplier=0,
                           allow_small_or_imprecise_dtypes=True)
            # small tables go through the Act queue: the SP queue must issue
            # the first x DMA with no queued predecessor
            gs_all = cp.tile([128, nreg], BF16)
            nc.scalar.dma_start(out=gs_all[:], in_=gsd[:])
            idx_t = cp.tile([128, 8], mybir.dt.int16)
            nc.scalar.dma_start(out=idx_t[:], in_=idxd[:])
            stub_t = cp.tile([128, w3 // CH], BF16)

            # early descriptor generation for the final store (reads the
            # untracked alias, so it runs as soon as the idx table lands)
            nc.gpsimd.dma_scatter_add(
                outl[:, 0:w3], stage_lr[:, None, :], idx_t[:], 128, 128, w3,
                elem_step=step3, prepare_only=True,
                sem=nc.alloc_semaphore("swdge_out"))

            # staging tiles for the non-final scatter groups
            stages = [stgpool.tile([128, smax, CH], BF16, name=f"stage{g}")
                      for g in range(nscat - 1)]

            # input DMA schedule: XPC chunks per DMA over the regular chunks,
            # special chunk last on its own
            dma_plan = {}
            kk = 0
            while kk < nreg:
                dma_plan[kk] = min(XPC, nreg - kk)
                kk += dma_plan[kk]
            dma_plan[nreg] = 1

            # all onehots (group -> slot) generated upfront on the DVE while
            # the input stream is still filling; steady-state DVE then does
            # only the fold work and never falls behind the arrival rate
            ohs = []
            kbase = 0
            for g, sg in enumerate(groups):
                oh_t = ohpool.tile([128, sg, 128], BF16, name=f"oh{g}")
                nc.vector.tensor_tensor(
                    out=oh_t[:],
                    in0=gs_all[:, kbase:kbase + sg, None]
                        .to_broadcast([128, sg, 128]),
                    in1=iota_t[:, None, :].to_broadcast([128, sg, 128]),
                    op=mybir.AluOpType.is_equal)
                ohs.append(oh_t)
                kbase += sg

            k = 0
            x_t = None
            xoff = 0
            for g, sg in enumerate(groups):
                oh_t = ohs[g]
                last = g == nscat - 1
                for kl in range(sg):
                    if k in dma_plan:
                        nxp = dma_plan[k]
                        x_t = xpool.tile([128, XPC, G * CH], BF16)
                        nc.sync.dma_start(
                            out=x_t[:, 0:nxp],
                            in_=xd[k:k + nxp].rearrange("k p c -> p k c"))
                        xoff = k
                    v = x_t[:, k - xoff]
                    # 2-level bf16 fold tree (members 16 -> 4 quarter-sums)
                    nc.vector.tensor_add(out=v[0:128, 0:8 * CH],
                                         in0=v[0:128, 0:8 * CH],
                                         in1=v[0:128, 8 * CH:16 * CH])
                    nc.vector.tensor_add(out=v[0:128, 0:4 * CH],
                                         in0=v[0:128, 0:4 * CH],
                                         in1=v[0:128, 4 * CH:8 * CH])
                    # the PE finishes the reduction: 4 accumulating matmuls
                    # sum the quarter-sums of every group into their voxel
                    # slot, all in f32 PSUM
                    ps_t = pspool.tile([128, CH], F32)
                    for j in range(4):
                        nc.tensor.matmul(out=ps_t[:], lhsT=oh_t[:, kl, :],
                                         rhs=v[0:128, j * CH:(j + 1) * CH],
                                         start=(j == 0), stop=(j == 3))
                    # Act downconverts PSUM f32 -> bf16 staging
                    dst = stage_l[:, kl * CH:(kl + 1) * CH] if last \
                        else stages[g][:, kl, :]
                    nc.scalar.copy(out=dst, in_=ps_t[:])
                    k += 1
                if not last:
                    nc.scalar.dma_start(out=outc[g][:, 0:sg, :],
                                        in_=stages[g][:, 0:sg, :])

            # --- special chunk: 128 single-group voxels, folded by the
            # otherwise-idle DVE straight into the last staging tile ---
            xs_t = xpool.tile([128, XPC, G * CH], BF16)
            nc.sync.dma_start(out=xs_t[:, 0:1],
                              in_=xd[nreg:ngc].rearrange("k p c -> p k c"))
            vs = xs_t[:, 0]
            sgl = groups[-1]
            nc.vector.tensor_add(out=vs[0:128, 0:8 * CH],
                                 in0=vs[0:128, 0:8 * CH],
                                 in1=vs[0:128, 8 * CH:16 * CH])
            nc.vector.tensor_add(out=vs[0:128, 0:4 * CH],
                                 in0=vs[0:128, 0:4 * CH],
                                 in1=vs[0:128, 4 * CH:8 * CH])
            nc.vector.tensor_add(out=vs[0:128, 0:2 * CH],
                                 in0=vs[0:128, 0:2 * CH],
                                 in1=vs[0:128, 2 * CH:4 * CH])
            nc.vector.tensor_add(out=stage_l[:, sgl * CH:(sgl + 1) * CH],
                                 in0=vs[0:128, 0:CH],
                                 in1=vs[0:128, CH:2 * CH])

            # fence: a cheap Pool add touching one column of every writer's
            # slice of stage_l makes the trigger (gated on the Pool engine
            # tick) wait for all staging writes; the scatter's own source
            # read goes through the untracked alias
            stub_view = stage_l[:].rearrange("p (s c) -> p s c", c=CH)[:, :, 0]
            nc.gpsimd.tensor_tensor(out=stub_t[:], in0=stub_view,
                                    in1=stub_view, op=mybir.AluOpType.add)
            nc.gpsimd.trigger_dma(count=None)
    nc.finalize()

    # The prep's completion (+16 at trigger-fired DMA completion) must land
    # on the tile-scheduled DMASW lane sem the drain barrier waits on; the
    # manual sem= occupied the descriptor's single sem slot, so retarget it.
    fn = nc.m.functions[0]
    prep = None
    dmasw = None
    for blk in fn.blocks:
        for ins in blk.instructions:
            if "ScatterAdd" in type(ins).__name__:
                prep = ins
            if ins.sync_info:
                for w in ins.sync_info.on_wait:
                    if w.ant_name and w.ant_name.startswith("DMASW"):
                        dmasw = w
    assert prep is not None and dmasw is not None
    u = prep.sync_info.on_update[0]
    u.ant_name = dmasw.ant_name
    u.id = dmasw.id
    return nc


def kernel(x, rots, trans, intrins, post_rots, post_trans):
    from concourse.bass_utils import run_bass_kernel_spmd

    x = np.asarray(x, dtype=np.float32)
    flat, kept = _geometry_rows(rots, trans, intrins, post_rots, post_trans)
    ngc, gather, gslot, rows, sp_gather, sp_rows = _plan(flat, kept)
    nreg = ngc - 1
    groups = _split_groups(nreg)
    nscat = len(groups)

    xf = x.reshape(NP, CH).astype(BF)
    xf_ext = np.concatenate([xf, np.zeros((1, CH), BF)], axis=0)

    # identity scatter indices, [16, 8] wrap replicated across partitions
    t = np.arange(128)
    idx16 = np.zeros((16, 8), np.int16)
    idx16[t % 16, t // 16] = t.astype(np.int16)
    idxv = np.ascontiguousarray(np.tile(idx16, (8, 1)))

    in_maps = []
    for core in range(NCORES):
        gidx = gather[core * nreg:(core + 1) * nreg]        # [nreg,128,G]
        xdv = np.empty((ngc, 128, G * CH), BF)
        xdv[:nreg] = xf_ext[gidx].reshape(nreg, 128, G * CH)  # member-major
        # special chunk: one single-group voxel per partition, member-major
        xdv[nreg] = xf_ext[sp_gather[core]].reshape(128, G * CH)
        gs = np.ascontiguousarray(
            gslot[core * nreg:(core + 1) * nreg].T.astype(BF))  # [128,nreg]
        in_maps.append(dict(xd=xdv, gsd=gs, idxd=idxv))

    if ngc not in _CACHE:
        _CACHE[ngc] = _build_kernel(ngc)
    nc = _CACHE[ngc]
    _CACHE["last_nc"] = nc

    res = run_bass_kernel_spmd(nc, in_maps, core_ids=list(range(NCORES)))

    # host-side scatter of compact voxel sums into the dense BEV grid
    nearly = sum(groups[:-1])
    kl_of_chunk = np.concatenate(
        [np.arange(sg) for sg in groups[:-1]])               # [nearly]
    g_of_chunk = np.repeat(np.arange(nscat - 1), groups[:-1])
    pooled = np.zeros((NROWS, CH), np.float32)
    for core in range(NCORES):
        o = np.asarray(res.results[core]["outc"], dtype=np.float32)
        ol = np.asarray(res.results[core]["outl"], dtype=np.float32)
        ol = ol[:, :(groups[-1] + 1) * CH]
        chunk_vals = np.empty((nreg, 128, CH), np.float32)
        chunk_vals[:nearly] = o[g_of_chunk, :, kl_of_chunk]
        lvals = ol.reshape(128, groups[-1] + 1, CH).transpose(1, 0, 2)
        chunk_vals[nearly:nreg] = lvals[:groups[-1]]
        r = rows[core * nreg:(core + 1) * nreg]              # [nreg,128]
        m = r >= 0
        pooled[r[m]] = chunk_vals[m]
        sp_vals = lvals[groups[-1]]                          # special column
        rs = sp_rows[core]
        ms = rs >= 0
        pooled[rs[ms]] = sp_vals[ms]
    final = pooled.reshape(B, NZ, NY, NX, CH) \
        .transpose(0, 1, 4, 2, 3).reshape(B, NZ * CH, NY, NX)
    return final


# revision 40
# speedup vs baseline: 2.1661x; 1.0138x over previous
"""Trainium2 Bass kernel for LiftSplatShoot voxel pooling (segment_reduce).

kernel(**inputs) takes the FULL inputs and returns the FULL output
(B, NZ*C, NY, NX) float32.

Strategy (8 NeuronCores, globally balanced, fully disjoint):
  host: replicate the reference geometry with eager jnp ops on CPU
        (bit-identical voxel assignment), sort all kept points by dense
        output row, round every voxel run up to whole groups of 16
        members, bin-pack voxel runs (FFD) into chunks of <=128 groups,
        split the chunk list evenly across the 8 cores, and pre-gather x
        (cast to bf16) into the device layout [chunk, group, member, ch].
  device (SPMD, per chunk):
        DMA x tile [128 groups, 16mem x 64ch] bf16; the PE does the whole
        reduction: 16 accumulating matmuls (one per member column block)
        with a bf16 onehot(group->slot) lhsT generated on the Pool engine
        -> PSUM [128 slots, 64] f32 voxel sums; the Act engine copies
        PSUM to bf16 SBUF staging; one DMA per scatter-group of chunks
        writes the staged slot sums to DRAM (compact layout, no scatter).
        The last chunk holds 128 reserved single-group voxels and is
        folded by the otherwise-idle DVE straight into the final staging
        tile, keeping the drain tail short.
  host: scatter the compact per-(chunk,slot) voxel sums into the dense
        BEV grid (pure indexing, each voxel written exactly once), then
        transpose to (B, NZ*C, NY, NX) float32.
"""
import numpy as np
import ml_dtypes

BF = ml_dtypes.bfloat16

# ---- static problem config (hardcoded per contest rules) ----
B, N, C, D = 4, 4, 64, 41
OGH, OGW, DS = 256, 704, 16
FH, FW = OGH // DS, OGW // DS  # 16, 44
XB = (-51.2, 51.2, 0.4)
YB = (-51.2, 51.2, 0.4)
ZB = (-10.0, 10.0, 20.0)
NX, NY, NZ = 256, 256, 1
NP = B * N * D * FH * FW
NROWS = B * NZ * NY * NX

CH = 64     # channels per point row
G = 16      # members per group
NCORES = 8
SMAXG = 7   # max chunks per output staging group
XPC = 2     # chunks per input DMA
NFOLD = 1   # DVE fold levels (PE does 16>>NFOLD matmuls)
OH_UPFRONT = False

_CACHE = {}


def _geometry_rows(rots, trans, intrins, post_rots, post_trans):
    """Replicate reference geometry exactly (same eager jnp ops) and return
    the global flat voxel index per point and the kept mask (numpy).

    Runs on the jax CPU backend: the axon/neuron backend cannot lower
    jnp.linalg.inv (triangular-solve unsupported), and the grading reference
    must therefore run on CPU as well — matching its numerics bit-for-bit.
    """
    import jax
    cpu = jax.local_devices(backend="cpu")[0]
    with jax.default_device(cpu):
        return _geometry_rows_impl(rots, trans, intrins, post_rots, post_trans)


def _geometry_rows_impl(rots, trans, intrins, post_rots, post_trans):
    import jax.numpy as jnp
    rots = jnp.asarray(rots)
    trans = jnp.asarray(trans)
    intrins = jnp.asarray(intrins)
    post_rots = jnp.asarray(post_rots)
    post_trans = jnp.asarray(post_trans)

    dx = jnp.array([XB[2], YB[2], ZB[2]], jnp.float32)
    bx = jnp.array([XB[0] + XB[2] / 2.0, YB[0] + YB[2] / 2.0,
                    ZB[0] + ZB[2] / 2.0], jnp.float32)
    ds = (2.0 + jnp.arange(D, dtype=jnp.float32)).reshape(D, 1, 1) \
        * jnp.ones((1, FH, FW), jnp.float32)
    xs = jnp.linspace(0.0, OGW - 1, FW, dtype=jnp.float32).reshape(1, 1, FW) \
        * jnp.ones((D, FH, 1), jnp.float32)
    ys = jnp.linspace(0.0, OGH - 1, FH, dtype=jnp.float32).reshape(1, FH, 1) \
        * jnp.ones((D, 1, FW), jnp.float32)
    frustum = jnp.stack([xs, ys, ds], -1)

    pts = frustum[None, None] - post_trans[:, :, None, None, None, :]
    pts = jnp.einsum('bnij,bndhwj->bndhwi', jnp.linalg.inv(post_rots), pts)
    pts = jnp.concatenate([pts[..., :2] * pts[..., 2:3], pts[..., 2:3]], -1)
    combine = rots @ jnp.linalg.inv(intrins)
    geom = jnp.einsum('bnij,bndhwj->bndhwi', combine, pts) \
        + trans[:, :, None, None, None, :]

    vox = jnp.floor((geom.reshape(NP, 3) - (bx - dx / 2.0)) / dx).astype(jnp.int32)
    vox = np.asarray(vox)
    kept = (vox[:, 0] >= 0) & (vox[:, 0] < NX) & (vox[:, 1] >= 0) \
        & (vox[:, 1] < NY) & (vox[:, 2] >= 0) & (vox[:, 2] < NZ)
    bix = np.repeat(np.arange(B, dtype=np.int64), NP // B)
    flat = ((bix * NZ + vox[:, 2].astype(np.int64)) * NY + vox[:, 1]) * NX + vox[:, 0]
    return flat, kept


def _plan(flat, kept):
    """Bin-pack voxel runs into chunks of <=128 groups of <=16 members.

    Per core, the LAST chunk is "special": 128 reserved single-group voxels,
    folded by the DVE without the matmul machinery (short drain tail).
    Returns (ngc, gather, gslot, rows, sp_gather, sp_rows) where
      gather [NREG_TOT, 128, G] int64: point index per member (NP = zero row)
      gslot  [NREG_TOT, 128] uint8: output slot per group partition
      rows   [NREG_TOT, 128] int64: global dense row per slot (-1 unused)
      sp_gather [NCORES, 128, G] int64: special-chunk member indices
      sp_rows   [NCORES, 128] int64: special-chunk dense rows (-1 unused)
    NREG_TOT = NCORES * (ngc - 1); core c owns regular chunks
    [c*(ngc-1), (c+1)*(ngc-1)) plus its special chunk.
    """
    idx = np.nonzero(kept)[0]
    rows_k = flat[idx]
    order = np.argsort(rows_k, kind="stable")
    idx = idx[order]
    rows_k = rows_k[order]
    uniq, starts, counts = np.unique(rows_k, return_index=True,
                                     return_counts=True)
    nvox = len(uniq)
    ngroups = (-(-counts // G)).astype(np.int64)
    assert ngroups.max() <= 128, f"voxel needs {ngroups.max()} groups"

    # reserve 128 single-group voxels per core for the special chunks
    single = np.nonzero(ngroups == 1)[0]
    assert len(single) >= NCORES * 128, "not enough single-group voxels"
    reserved = single[:NCORES * 128]
    is_res = np.zeros(nvox, bool)
    is_res[reserved] = True

    total_groups = int(ngroups[~is_res].sum())

    # First-fit-decreasing bin packing of voxel runs into 128-group chunks.
    vorder = np.argsort(-ngroups, kind="stable")
    vorder = vorder[~is_res[vorder]]
    nbins = -(-total_groups // 128)
    nbins = max(nbins, 1)
    while True:
        caps = np.full(nbins, 128, np.int64)
        vbin = np.full(nvox, -1, np.int64)
        ok = True
        for v in vorder:
            g = ngroups[v]
            fits = np.nonzero(caps >= g)[0]
            if len(fits) == 0:
                ok = False
                break
            b = fits[0]
            vbin[v] = b
            caps[b] -= g
        if ok:
            break
        nbins += 1

    nreg = -(-nbins // NCORES)          # regular chunks per core
    ngc = nreg + 1
    ntot = NCORES * nreg

    gather = np.full((ntot, 128, G), NP, np.int64)
    gslot = np.zeros((ntot, 128), np.uint8)
    rows = np.full((ntot, 128), -1, np.int64)

    # assign slots/partitions within each bin in voxel order
    binslot = np.zeros(nbins, np.int64)   # next slot id per bin
    binpart = np.zeros(nbins, np.int64)   # next group partition per bin
    for v in range(nvox):
        b = vbin[v]
        if b < 0:
            continue
        s = binslot[b]
        binslot[b] += 1
        rows[b, s] = uniq[v]
        c = int(counts[v])
        st = int(starts[v])
        ng = int(ngroups[v])
        p0 = binpart[b]
        binpart[b] += ng
        for j in range(ng):
            lo = st + j * G
            hi = st + min((j + 1) * G, c)
            gather[b, p0 + j, :hi - lo] = idx[lo:hi]
            gslot[b, p0 + j] = s
    assert binslot.max() <= 128 and binpart.max() <= 128

    sp_gather = np.full((NCORES, 128, G), NP, np.int64)
    sp_rows = np.full((NCORES, 128), -1, np.int64)
    for c in range(NCORES):
        vs = reserved[c * 128:(c + 1) * 128]
        for s, v in enumerate(vs):
            cnt = int(counts[v])
            st = int(starts[v])
            sp_gather[c, s, :cnt] = idx[st:st + cnt]
            sp_rows[c, s] = uniq[v]
    return ngc, gather, gslot, rows, sp_gather, sp_rows


def _split_groups(nreg):
    nscat = -(-nreg // SMAXG)
    base = nreg // nscat
    rem = nreg - base * nscat
    return [base + (1 if i < rem else 0) for i in range(nscat)]


def _build_kernel(ngc):
    import concourse.bacc as bacc
    import concourse.mybir as mybir
    import concourse.tile as tile
    F32 = mybir.dt.float32
    BF16 = mybir.dt.bfloat16

    nreg = ngc - 1
    groups = _split_groups(nreg)
    nscat = len(groups)
    smax = max(groups)
    w3 = (groups[-1] + 1) * CH          # last stage incl. special column

    step3 = -(-w3 // 128) * 128         # scatter stride must be 256B-aligned

    nc = bacc.Bacc("TRN2", target_bir_lowering=False, debug=False,
                   num_devices=NCORES)
    xd = nc.dram_tensor("xd", [ngc, 128, G * CH], BF16, kind="ExternalInput")
    gsd = nc.dram_tensor("gsd", [128, nreg], BF16, kind="ExternalInput")
    idxd = nc.dram_tensor("idxd", [128, 8], mybir.dt.int16,
                          kind="ExternalInput")
    outc = nc.dram_tensor("outc", [nscat - 1, 128, smax, CH], BF16,
                          kind="ExternalOutput")
    outl = nc.dram_tensor("outl", [128, step3], BF16, kind="ExternalOutput")

    # the final staging buffer lives in a hand-allocated arena with TWO
    # aliased handles: compute writes stage_l (tracked), while the early
    # SWDGE descriptor prep reads the alias — so its ~1us desc generation is
    # NOT ordered after the last write.  trigger_dma fires the transfer at
    # the end, skipping the ~1.3us HWDGE+DGE latency of a plain DMA; a tiny
    # Pool-engine read of stage_l fences the trigger behind all writes.
    arena = nc.alloc_sbuf_tensor("stage_arena", [128, w3], BF16)
    addr = nc.lookup_mloc(arena).addr
    stage_l = nc.alloc_sbuf_tensor_at("stage_l_w", [128, w3], BF16,
                                      offset=addr)
    stage_lr = nc.alloc_sbuf_tensor_at("stage_l_r", [128, w3], BF16,
                                       offset=addr)

    with tile.TileContext(nc) as tc, \
            nc.allow_low_precision(reason="bf16 voxel pooling"):
        with (
            tc.tile_pool(name="const", bufs=1) as cp,
            tc.tile_pool(name="xp", bufs=6) as xpool,
            tc.tile_pool(name="ohp", bufs=2) as ohpool,
            tc.tile_pool(name="psp", bufs=4, space="PSUM") as pspool,
            tc.tile_pool(name="stg", bufs=nscat) as stgpool,
        ):
            iota_t = cp.tile([128, 128], BF16)
            nc.gpsimd.iota(iota_t[:], pattern=[[1, 128]], base=0,
                           channel_multiplier=0,
                           allow_small_or_imprecise_dtypes=True)
            # small tables go through the Act queue: the SP queue must issue
            # the first x DMA with no queued predecessor
            gs_all = cp.tile([128, nreg], BF16)
            nc.scalar.dma_start(out=gs_all[:], in_=gsd[:])
            idx_t = cp.tile([128, 8], mybir.dt.int16)
            nc.scalar.dma_start(out=idx_t[:], in_=idxd[:])
            stub_t = cp.tile([128, w3 // CH], BF16)

            # early descriptor generation for the final store (reads the
            # untracked alias, so it runs as soon as the idx table lands)
            nc.gpsimd.dma_scatter_add(
                outl[:, 0:w3], stage_lr[:, None, :], idx_t[:], 128, 128, w3,
                elem_step=step3, prepare_only=True,
                sem=nc.alloc_semaphore("swdge_out"))

            # staging tiles for the non-final scatter groups
            stages = [stgpool.tile([128, smax, CH], BF16, name=f"stage{g}")
                      for g in range(nscat - 1)]

            # input DMA schedule: XPC chunks per DMA over the regular chunks,
            # special chunk last on its own
            dma_plan = {}
            kk = 0
            while kk < nreg:
                dma_plan[kk] = min(XPC, nreg - kk)
                kk += dma_plan[kk]
            dma_plan[nreg] = 1

            # onehots (group -> slot) on the DVE: either all upfront while
            # the input stream is still filling (steady-state DVE then does
            # only fold work), or interleaved per scatter group
            def gen_oh(g, sg, kbase):
                oh_t = ohpool.tile([128, sg, 128], BF16, name=f"oh{g}")
                nc.vector.tensor_tensor(
                    out=oh_t[:],
                    in0=gs_all[:, kbase:kbase + sg, None]
                        .to_broadcast([128, sg, 128]),
                    in1=iota_t[:, None, :].to_broadcast([128, sg, 128]),
                    op=mybir.AluOpType.is_equal)
                return oh_t

            ohs = {}
            if OH_UPFRONT:
                kbase = 0
                for g, sg in enumerate(groups):
                    ohs[g] = gen_oh(g, sg, kbase)
                    kbase += sg

            k = 0
            x_t = None
            xoff = 0
            for g, sg in enumerate(groups):
                oh_t = ohs[g] if OH_UPFRONT else gen_oh(g, sg, k)
                last = g == nscat - 1
                for kl in range(sg):
                    if k in dma_plan:
                        nxp = dma_plan[k]
                        x_t = xpool.tile([128, XPC, G * CH], BF16)
                        nc.sync.dma_start(
                            out=x_t[:, 0:nxp],
                            in_=xd[k:k + nxp].rearrange("k p c -> p k c"))
                        xoff = k
                    v = x_t[:, k - xoff]
                    # NFOLD-level bf16 fold tree on the DVE, then the PE
                    # finishes the reduction: accumulating matmuls sum the
                    # partial sums of every group into their voxel slot,
                    # all in f32 PSUM
                    for f in range(NFOLD):
                        half = (G >> (f + 1)) * CH
                        nc.vector.tensor_add(out=v[0:128, 0:half],
                                             in0=v[0:128, 0:half],
                                             in1=v[0:128, half:2 * half])
                    nmm = G >> NFOLD
                    ps_t = pspool.tile([128, CH], F32)
                    for j in range(nmm):
                        nc.tensor.matmul(out=ps_t[:], lhsT=oh_t[:, kl, :],
                                         rhs=v[0:128, j * CH:(j + 1) * CH],
                                         start=(j == 0), stop=(j == nmm - 1))
                    # Act downconverts PSUM f32 -> bf16 staging
                    dst = stage_l[:, kl * CH:(kl + 1) * CH] if last \
                        else stages[g][:, kl, :]
                    nc.scalar.copy(out=dst, in_=ps_t[:])
                    k += 1
                if not last:
                    nc.scalar.dma_start(out=outc[g][:, 0:sg, :],
                                        in_=stages[g][:, 0:sg, :])

            # --- special chunk: 128 single-group voxels, folded by the
            # otherwise-idle DVE straight into the last staging tile ---
            if nreg in dma_plan:
                xs_t = xpool.tile([128, XPC, G * CH], BF16)
                nc.sync.dma_start(out=xs_t[:, 0:1],
                                  in_=xd[nreg:ngc].rearrange("k p c -> p k c"))
                vs = xs_t[:, 0]
            else:
                vs = x_t[:, nreg - xoff]
            sgl = groups[-1]
            nc.vector.tensor_add(out=vs[0:128, 0:8 * CH],
                                 in0=vs[0:128, 0:8 * CH],
                                 in1=vs[0:128, 8 * CH:16 * CH])
            nc.vector.tensor_add(out=vs[0:128, 0:4 * CH],
                                 in0=vs[0:128, 0:4 * CH],
                                 in1=vs[0:128, 4 * CH:8 * CH])
            nc.vector.tensor_add(out=vs[0:128, 0:2 * CH],
                                 in0=vs[0:128, 0:2 * CH],
                                 in1=vs[0:128, 2 * CH:4 * CH])
            nc.vector.tensor_add(out=stage_l[:, sgl * CH:(sgl + 1) * CH],
                                 in0=vs[0:128, 0:CH],
                                 in1=vs[0:128, CH:2 * CH])

            # fence: a cheap Pool add touching one column of every writer's
            # slice of stage_l makes the trigger (gated on the Pool engine
            # tick) wait for all staging writes; the scatter's own source
            # read goes through the untracked alias
            stub_view = stage_l[:].rearrange("p (s c) -> p s c", c=CH)[:, :, 0]
            nc.gpsimd.tensor_tensor(out=stub_t[:], in0=stub_view,
                                    in1=stub_view, op=mybir.AluOpType.add)
            nc.gpsimd.trigger_dma(count=None)
    nc.finalize()

    # The prep's completion (+16 at trigger-fired DMA completion) must land
    # on the tile-scheduled DMASW lane sem the drain barrier waits on; the
    # manual sem= occupied the descriptor's single sem slot, so retarget it.
    fn = nc.m.functions[0]
    prep = None
    dmasw = None
    for blk in fn.blocks:
        for ins in blk.instructions:
            if "ScatterAdd" in type(ins).__name__:
                prep = ins
            if ins.sync_info:
                for w in ins.sync_info.on_wait:
                    if w.ant_name and w.ant_name.startswith("DMASW"):
                        dmasw = w
    assert prep is not None and dmasw is not None
    u = prep.sync_info.on_update[0]
    u.ant_name = dmasw.ant_name
    u.id = dmasw.id
    return nc


def kernel(x, rots, trans, intrins, post_rots, post_trans):
    from concourse.bass_utils import run_bass_kernel_spmd

    x = np.asarray(x, dtype=np.float32)
    flat, kept = _geometry_rows(rots, trans, intrins, post_rots, post_trans)
    ngc, gather, gslot, rows, sp_gather, sp_rows = _plan(flat, kept)
    nreg = ngc - 1
    groups = _split_groups(nreg)
    nscat = len(groups)

    xf = x.reshape(NP, CH).astype(BF)
    xf_ext = np.concatenate([xf, np.zeros((1, CH), BF)], axis=0)

    # identity scatter indices, [16, 8] wrap replicated across partitions
    t = np.arange(128)
    idx16 = np.zeros((16, 8), np.int16)
    idx16[t % 16, t // 16] = t.astype(np.int16)
    idxv = np.ascontiguousarray(np.tile(idx16, (8, 1)))

    in_maps = []
    for core in range(NCORES):
        gidx = gather[core * nreg:(core + 1) * nreg]        # [nreg,128,G]
        xdv = np.empty((ngc, 128, G * CH), BF)
        xdv[:nreg] = xf_ext[gidx].reshape(nreg, 128, G * CH)  # member-major
        # special chunk: one single-group voxel per partition, member-major
        xdv[nreg] = xf_ext[sp_gather[core]].reshape(128, G * CH)
        gs = np.ascontiguousarray(
            gslot[core * nreg:(core + 1) * nreg].T.astype(BF))  # [128,nreg]
        in_maps.append(dict(xd=xdv, gsd=gs, idxd=idxv))

    if ngc not in _CACHE:
        _CACHE[ngc] = _build_kernel(ngc)
    nc = _CACHE[ngc]
    _CACHE["last_nc"] = nc

    res = run_bass_kernel_spmd(nc, in_maps, core_ids=list(range(NCORES)))

    # host-side scatter of compact voxel sums into the dense BEV grid
    nearly = sum(groups[:-1])
    kl_of_chunk = np.concatenate(
        [np.arange(sg) for sg in groups[:-1]])               # [nearly]
    g_of_chunk = np.repeat(np.arange(nscat - 1), groups[:-1])
    pooled = np.zeros((NROWS, CH), np.float32)
    for core in range(NCORES):
        o = np.asarray(res.results[core]["outc"], dtype=np.float32)
        ol = np.asarray(res.results[core]["outl"], dtype=np.float32)
        ol = ol[:, :(groups[-1] + 1) * CH]
        chunk_vals = np.empty((nreg, 128, CH), np.float32)
        chunk_vals[:nearly] = o[g_of_chunk, :, kl_of_chunk]
        lvals = ol.reshape(128, groups[-1] + 1, CH).transpose(1, 0, 2)
        chunk_vals[nearly:nreg] = lvals[:groups[-1]]
        r = rows[core * nreg:(core + 1) * nreg]              # [nreg,128]
        m = r >= 0
        pooled[r[m]] = chunk_vals[m]
        sp_vals = lvals[groups[-1]]                          # special column
        rs = sp_rows[core]
        ms = rs >= 0
        pooled[rs[ms]] = sp_vals[ms]
    final = pooled.reshape(B, NZ, NY, NX, CH) \
        .transpose(0, 1, 4, 2, 3).reshape(B, NZ * CH, NY, NX)
    return final


# revision 46
# speedup vs baseline: 2.1857x; 1.0091x over previous
"""Trainium2 Bass kernel for LiftSplatShoot voxel pooling (segment_reduce).

kernel(**inputs) takes the FULL inputs and returns the FULL output
(B, NZ*C, NY, NX) float32.

Strategy (8 NeuronCores, globally balanced, fully disjoint):
  host: replicate the reference geometry with eager jnp ops on CPU
        (bit-identical voxel assignment), sort all kept points by dense
        output row, round every voxel run up to whole groups of 16
        members, bin-pack voxel runs (FFD) into chunks of <=128 groups,
        split the chunk list evenly across the 8 cores, and pre-gather x
        (cast to bf16) into the device layout [chunk, group, member, ch].
  device (SPMD, per chunk):
        DMA x tile [128 groups, 16mem x 64ch] bf16; the PE does the whole
        reduction: 16 accumulating matmuls (one per member column block)
        with a bf16 onehot(group->slot) lhsT generated on the Pool engine
        -> PSUM [128 slots, 64] f32 voxel sums; the Act engine copies
        PSUM to bf16 SBUF staging; one DMA per scatter-group of chunks
        writes the staged slot sums to DRAM (compact layout, no scatter).
        The last chunk holds 128 reserved single-group voxels and is
        folded by the otherwise-idle DVE straight into the final staging
        tile, keeping the drain tail short.
  host: scatter the compact per-(chunk,slot) voxel sums into the dense
        BEV grid (pure indexing, each voxel written exactly once), then
        transpose to (B, NZ*C, NY, NX) float32.
"""
import numpy as np
import ml_dtypes

BF = ml_dtypes.bfloat16

# ---- static problem config (hardcoded per contest rules) ----
B, N, C, D = 4, 4, 64, 41
OGH, OGW, DS = 256, 704, 16
FH, FW = OGH // DS, OGW // DS  # 16, 44
XB = (-51.2, 51.2, 0.4)
YB = (-51.2, 51.2, 0.4)
ZB = (-10.0, 10.0, 20.0)
NX, NY, NZ = 256, 256, 1
NP = B * N * D * FH * FW
NROWS = B * NZ * NY * NX

CH = 64     # channels per point row
G = 16      # members per group
NCORES = 8
SMAXG = 7   # max chunks per output staging group
XPC = 2     # chunks per input DMA
NFOLD = 1   # DVE fold levels (PE does 16>>NFOLD matmuls)
OH_UPFRONT = False

_CACHE = {}


def _geometry_rows(rots, trans, intrins, post_rots, post_trans):
    """Replicate reference geometry exactly (same eager jnp ops) and return
    the global flat voxel index per point and the kept mask (numpy).

    Runs on the jax CPU backend: the axon/neuron backend cannot lower
    jnp.linalg.inv (triangular-solve unsupported), and the grading reference
    must therefore run on CPU as well — matching its numerics bit-for-bit.
    """
    import jax
    cpu = jax.local_devices(backend="cpu")[0]
    with jax.default_device(cpu):
        return _geometry_rows_impl(rots, trans, intrins, post_rots, post_trans)


def _geometry_rows_impl(rots, trans, intrins, post_rots, post_trans):
    import jax.numpy as jnp
    rots = jnp.asarray(rots)
    trans = jnp.asarray(trans)
    intrins = jnp.asarray(intrins)
    post_rots = jnp.asarray(post_rots)
    post_trans = jnp.asarray(post_trans)

    dx = jnp.array([XB[2], YB[2], ZB[2]], jnp.float32)
    bx = jnp.array([XB[0] + XB[2] / 2.0, YB[0] + YB[2] / 2.0,
                    ZB[0] + ZB[2] / 2.0], jnp.float32)
    ds = (2.0 + jnp.arange(D, dtype=jnp.float32)).reshape(D, 1, 1) \
        * jnp.ones((1, FH, FW), jnp.float32)
    xs = jnp.linspace(0.0, OGW - 1, FW, dtype=jnp.float32).reshape(1, 1, FW) \
        * jnp.ones((D, FH, 1), jnp.float32)
    ys = jnp.linspace(0.0, OGH - 1, FH, dtype=jnp.float32).reshape(1, FH, 1) \
        * jnp.ones((D, 1, FW), jnp.float32)
    frustum = jnp.stack([xs, ys, ds], -1)

    pts = frustum[None, None] - post_trans[:, :, None, None, None, :]
    pts = jnp.einsum('bnij,bndhwj->bndhwi', jnp.linalg.inv(post_rots), pts)
    pts = jnp.concatenate([pts[..., :2] * pts[..., 2:3], pts[..., 2:3]], -1)
    combine = rots @ jnp.linalg.inv(intrins)
    geom = jnp.einsum('bnij,bndhwj->bndhwi', combine, pts) \
        + trans[:, :, None, None, None, :]

    vox = jnp.floor((geom.reshape(NP, 3) - (bx - dx / 2.0)) / dx).astype(jnp.int32)
    vox = np.asarray(vox)
    kept = (vox[:, 0] >= 0) & (vox[:, 0] < NX) & (vox[:, 1] >= 0) \
        & (vox[:, 1] < NY) & (vox[:, 2] >= 0) & (vox[:, 2] < NZ)
    bix = np.repeat(np.arange(B, dtype=np.int64), NP // B)
    flat = ((bix * NZ + vox[:, 2].astype(np.int64)) * NY + vox[:, 1]) * NX + vox[:, 0]
    return flat, kept


def _plan(flat, kept):
    """Bin-pack voxel runs into chunks of <=128 groups of <=16 members.

    Per core, the LAST chunk is "special": 128 reserved single-group voxels,
    folded by the DVE without the matmul machinery (short drain tail).
    Returns (ngc, gather, gslot, rows, sp_gather, sp_rows) where
      gather [NREG_TOT, 128, G] int64: point index per member (NP = zero row)
      gslot  [NREG_TOT, 128] uint8: output slot per group partition
      rows   [NREG_TOT, 128] int64: global dense row per slot (-1 unused)
      sp_gather [NCORES, 128, G] int64: special-chunk member indices
      sp_rows   [NCORES, 128] int64: special-chunk dense rows (-1 unused)
    NREG_TOT = NCORES * (ngc - 1); core c owns regular chunks
    [c*(ngc-1), (c+1)*(ngc-1)) plus its special chunk.
    """
    idx = np.nonzero(kept)[0]
    rows_k = flat[idx]
    order = np.argsort(rows_k, kind="stable")
    idx = idx[order]
    rows_k = rows_k[order]
    uniq, starts, counts = np.unique(rows_k, return_index=True,
                                     return_counts=True)
    nvox = len(uniq)
    ngroups = (-(-counts // G)).astype(np.int64)
    assert ngroups.max() <= 128, f"voxel needs {ngroups.max()} groups"

    # reserve 128 single-group voxels per core for the special chunks
    single = np.nonzero(ngroups == 1)[0]
    assert len(single) >= NCORES * 128, "not enough single-group voxels"
    reserved = single[:NCORES * 128]
    is_res = np.zeros(nvox, bool)
    is_res[reserved] = True

    total_groups = int(ngroups[~is_res].sum())

    # First-fit-decreasing bin packing of voxel runs into 128-group chunks.
    vorder = np.argsort(-ngroups, kind="stable")
    vorder = vorder[~is_res[vorder]]

    def ffd(nb, allow_overflow):
        caps = np.full(max(nb, 1), 128, np.int64)
        vb = np.full(nvox, -1, np.int64)
        over = []
        for v in vorder:
            g = ngroups[v]
            fits = np.nonzero(caps >= g)[0]
            if len(fits) == 0:
                if not allow_overflow:
                    return None, None
                over.append(int(v))
                continue
            b = fits[0]
            vb[v] = b
            caps[b] -= g
        return vb, over

    nbins = max(-(-total_groups // 128), 1)
    while True:
        vbin, _ = ffd(nbins, False)
        if vbin is not None:
            break
        nbins += 1

    # If nbins doesn't divide evenly over the cores, move the remainder into
    # 8 balanced PARTIAL bins (one per core, transferred only up to pcap
    # group partitions) instead of padding with empty full-size chunks.
    pcap = 0
    pvox = [[] for _ in range(NCORES)]
    nfull = nbins - nbins % NCORES
    if nbins % NCORES and nfull > 0:
        vbin2, over = ffd(nfull, True)
        over.sort(key=lambda v: -int(ngroups[v]))
        pload = np.zeros(NCORES, np.int64)
        ok = True
        for v in over:
            c = int(np.argmin(pload))
            if pload[c] + ngroups[v] > 120:
                ok = False
                break
            pvox[c].append(v)
            pload[c] += ngroups[v]
        if ok and len(over) > 0:
            pcap = int(-(-int(pload.max()) // 16) * 16)
            vbin = vbin2
            nbins = nfull
        else:
            pvox = [[] for _ in range(NCORES)]

    fullper = nbins // NCORES if pcap else -(-nbins // NCORES)
    nreg = fullper + (1 if pcap else 0)   # regular chunks per core
    ngc = nreg + 1
    ntot = NCORES * nreg

    gather = np.full((ntot, 128, G), NP, np.int64)
    gslot = np.zeros((ntot, 128), np.uint8)
    rows = np.full((ntot, 128), -1, np.int64)

    # map voxel -> global chunk id (core-major, partial last per core)
    vchunk = np.full(nvox, -1, np.int64)
    m = vbin >= 0
    vchunk[m] = (vbin[m] // fullper) * nreg + (vbin[m] % fullper)
    if pcap:
        for c in range(NCORES):
            for v in pvox[c]:
                vchunk[v] = c * nreg + (nreg - 1)

    # assign slots/partitions within each chunk in voxel order
    binslot = np.zeros(ntot, np.int64)   # next slot id per chunk
    binpart = np.zeros(ntot, np.int64)   # next group partition per chunk
    for v in range(nvox):
        b = vchunk[v]
        if b < 0:
            continue
        s = binslot[b]
        binslot[b] += 1
        rows[b, s] = uniq[v]
        c = int(counts[v])
        st = int(starts[v])
        ng = int(ngroups[v])
        p0 = binpart[b]
        binpart[b] += ng
        for j in range(ng):
            lo = st + j * G
            hi = st + min((j + 1) * G, c)
            gather[b, p0 + j, :hi - lo] = idx[lo:hi]
            gslot[b, p0 + j] = s
    assert binslot.max() <= 128 and binpart.max() <= 128
    if pcap:
        # untransferred partitions of the partial chunks hold stale SBUF
        # data; route them to dead slot 127 (host never reads it there)
        for c in range(NCORES):
            b = c * nreg + (nreg - 1)
            assert binslot[b] <= 120 and binpart[b] <= pcap
            gslot[b, binpart[b]:] = 127

    sp_gather = np.full((NCORES, 128, G), NP, np.int64)
    sp_rows = np.full((NCORES, 128), -1, np.int64)
    for c in range(NCORES):
        vs = reserved[c * 128:(c + 1) * 128]
        for s, v in enumerate(vs):
            cnt = int(counts[v])
            st = int(starts[v])
            sp_gather[c, s, :cnt] = idx[st:st + cnt]
            sp_rows[c, s] = uniq[v]
    return ngc, pcap, gather, gslot, rows, sp_gather, sp_rows


def _split_groups(nreg):
    nscat = -(-nreg // SMAXG)
    base = nreg // nscat
    rem = nreg - base * nscat
    return [base + (1 if i < rem else 0) for i in range(nscat)]


def _build_kernel(ngc, pcap=0):
    import concourse.bacc as bacc
    import concourse.mybir as mybir
    import concourse.tile as tile
    F32 = mybir.dt.float32
    BF16 = mybir.dt.bfloat16

    nreg = ngc - 1
    groups = _split_groups(nreg)
    nscat = len(groups)
    smax = max(groups)
    w3 = (groups[-1] + 1) * CH          # last stage incl. special column

    step3 = -(-w3 // 128) * 128         # scatter stride must be 256B-aligned

    nc = bacc.Bacc("TRN2", target_bir_lowering=False, debug=False,
                   num_devices=NCORES)
    xd = nc.dram_tensor("xd", [ngc, 128, G * CH], BF16, kind="ExternalInput")
    gsd = nc.dram_tensor("gsd", [128, nreg], BF16, kind="ExternalInput")
    idxd = nc.dram_tensor("idxd", [128, 8], mybir.dt.int16,
                          kind="ExternalInput")
    outc = nc.dram_tensor("outc", [nscat - 1, 128, smax, CH], BF16,
                          kind="ExternalOutput")
    outl = nc.dram_tensor("outl", [128, step3], BF16, kind="ExternalOutput")

    # the final staging buffer lives in a hand-allocated arena with TWO
    # aliased handles: compute writes stage_l (tracked), while the early
    # SWDGE descriptor prep reads the alias — so its ~1us desc generation is
    # NOT ordered after the last write.  trigger_dma fires the transfer at
    # the end, skipping the ~1.3us HWDGE+DGE latency of a plain DMA; a tiny
    # Pool-engine read of stage_l fences the trigger behind all writes.
    arena = nc.alloc_sbuf_tensor("stage_arena", [128, w3], BF16)
    addr = nc.lookup_mloc(arena).addr
    stage_l = nc.alloc_sbuf_tensor_at("stage_l_w", [128, w3], BF16,
                                      offset=addr)
    stage_lr = nc.alloc_sbuf_tensor_at("stage_l_r", [128, w3], BF16,
                                       offset=addr)

    with tile.TileContext(nc) as tc, \
            nc.allow_low_precision(reason="bf16 voxel pooling"):
        with (
            tc.tile_pool(name="const", bufs=1) as cp,
            tc.tile_pool(name="xp", bufs=6) as xpool,
            tc.tile_pool(name="ohp", bufs=2) as ohpool,
            tc.tile_pool(name="psp", bufs=4, space="PSUM") as pspool,
            tc.tile_pool(name="stg", bufs=nscat) as stgpool,
        ):
            iota_t = cp.tile([128, 128], BF16)
            nc.gpsimd.iota(iota_t[:], pattern=[[1, 128]], base=0,
                           channel_multiplier=0,
                           allow_small_or_imprecise_dtypes=True)
            # small tables go through the Act queue: the SP queue must issue
            # the first x DMA with no queued predecessor
            gs_all = cp.tile([128, nreg], BF16)
            nc.scalar.dma_start(out=gs_all[:], in_=gsd[:])
            idx_t = cp.tile([128, 8], mybir.dt.int16)
            nc.scalar.dma_start(out=idx_t[:], in_=idxd[:])
            stub_t = cp.tile([128, w3 // CH], BF16)

            # early descriptor generation for the final store (reads the
            # untracked alias, so it runs as soon as the idx table lands)
            nc.gpsimd.dma_scatter_add(
                outl[:, 0:w3], stage_lr[:, None, :], idx_t[:], 128, 128, w3,
                elem_step=step3, prepare_only=True,
                sem=nc.alloc_semaphore("swdge_out"))

            # staging tiles for the non-final scatter groups
            stages = [stgpool.tile([128, smax, CH], BF16, name=f"stage{g}")
                      for g in range(nscat - 1)]

            # input DMA schedule: XPC chunks per DMA over the full regular
            # chunks; the partial chunk (if any) and the special chunk go
            # last, each on its own
            nfullc = nreg - 1 if pcap else nreg
            dma_plan = {}
            kk = 0
            while kk < nfullc:
                dma_plan[kk] = min(XPC, nfullc - kk)
                kk += dma_plan[kk]
            if pcap:
                dma_plan[nreg - 1] = 1
            dma_plan[nreg] = 1

            # onehots (group -> slot) on the DVE: either all upfront while
            # the input stream is still filling (steady-state DVE then does
            # only fold work), or interleaved per scatter group
            def gen_oh(g, sg, kbase):
                oh_t = ohpool.tile([128, sg, 128], BF16, name=f"oh{g}")
                nc.vector.tensor_tensor(
                    out=oh_t[:],
                    in0=gs_all[:, kbase:kbase + sg, None]
                        .to_broadcast([128, sg, 128]),
                    in1=iota_t[:, None, :].to_broadcast([128, sg, 128]),
                    op=mybir.AluOpType.is_equal)
                return oh_t

            ohs = {}
            if OH_UPFRONT:
                kbase = 0
                for g, sg in enumerate(groups):
                    ohs[g] = gen_oh(g, sg, kbase)
                    kbase += sg

            k = 0
            x_t = None
            xoff = 0
            for g, sg in enumerate(groups):
                oh_t = ohs[g] if OH_UPFRONT else gen_oh(g, sg, k)
                last = g == nscat - 1
                for kl in range(sg):
                    if k in dma_plan:
                        nxp = dma_plan[k]
                        x_t = xpool.tile([128, XPC, G * CH], BF16)
                        if pcap and k == nreg - 1:
                            # partial chunk: only pcap group partitions are
                            # real; the rest hold stale data that the dead
                            # slot-127 onehot column zeroes out of every sum
                            nc.sync.dma_start(
                                out=x_t[0:pcap, 0:1],
                                in_=xd[k:k + 1]
                                .rearrange("k p c -> p k c")[0:pcap])
                        else:
                            nc.sync.dma_start(
                                out=x_t[:, 0:nxp],
                                in_=xd[k:k + nxp].rearrange("k p c -> p k c"))
                        xoff = k
                    v = x_t[:, k - xoff]
                    # NFOLD-level bf16 fold tree on the DVE, then the PE
                    # finishes the reduction: accumulating matmuls sum the
                    # partial sums of every group into their voxel slot,
                    # all in f32 PSUM
                    for f in range(NFOLD):
                        half = (G >> (f + 1)) * CH
                        nc.vector.tensor_add(out=v[0:128, 0:half],
                                             in0=v[0:128, 0:half],
                                             in1=v[0:128, half:2 * half])
                    nmm = G >> NFOLD
                    ps_t = pspool.tile([128, CH], F32)
                    for j in range(nmm):
                        nc.tensor.matmul(out=ps_t[:], lhsT=oh_t[:, kl, :],
                                         rhs=v[0:128, j * CH:(j + 1) * CH],
                                         start=(j == 0), stop=(j == nmm - 1))
                    # Act downconverts PSUM f32 -> bf16 staging
                    dst = stage_l[:, kl * CH:(kl + 1) * CH] if last \
                        else stages[g][:, kl, :]
                    nc.scalar.copy(out=dst, in_=ps_t[:])
                    k += 1
                if not last:
                    nc.scalar.dma_start(out=outc[g][:, 0:sg, :],
                                        in_=stages[g][:, 0:sg, :])

            # --- special chunk: 128 single-group voxels, folded by the
            # otherwise-idle DVE straight into the last staging tile ---
            if nreg in dma_plan:
                xs_t = xpool.tile([128, XPC, G * CH], BF16)
                nc.sync.dma_start(out=xs_t[:, 0:1],
                                  in_=xd[nreg:ngc].rearrange("k p c -> p k c"))
                vs = xs_t[:, 0]
            else:
                vs = x_t[:, nreg - xoff]
            sgl = groups[-1]
            nc.vector.tensor_add(out=vs[0:128, 0:8 * CH],
                                 in0=vs[0:128, 0:8 * CH],
                                 in1=vs[0:128, 8 * CH:16 * CH])
            nc.vector.tensor_add(out=vs[0:128, 0:4 * CH],
                                 in0=vs[0:128, 0:4 * CH],
                                 in1=vs[0:128, 4 * CH:8 * CH])
            nc.vector.tensor_add(out=vs[0:128, 0:2 * CH],
                                 in0=vs[0:128, 0:2 * CH],
                                 in1=vs[0:128, 2 * CH:4 * CH])
            nc.vector.tensor_add(out=stage_l[:, sgl * CH:(sgl + 1) * CH],
                                 in0=vs[0:128, 0:CH],
                                 in1=vs[0:128, CH:2 * CH])

            # fence: a cheap Pool add touching one column of every writer's
            # slice of stage_l makes the trigger (gated on the Pool engine
            # tick) wait for all staging writes; the scatter's own source
            # read goes through the untracked alias
            stub_view = stage_l[:].rearrange("p (s c) -> p s c", c=CH)[:, :, 0]
            nc.gpsimd.tensor_tensor(out=stub_t[:], in0=stub_view,
                                    in1=stub_view, op=mybir.AluOpType.add)
            nc.gpsimd.trigger_dma(count=None)
    nc.finalize()

    # The prep's completion (+16 at trigger-fired DMA completion) must land
    # on the tile-scheduled DMASW lane sem the drain barrier waits on; the
    # manual sem= occupied the descriptor's single sem slot, so retarget it.
    fn = nc.m.functions[0]
    prep = None
    dmasw = None
    for blk in fn.blocks:
        for ins in blk.instructions:
            if "ScatterAdd" in type(ins).__name__:
                prep = ins
            if ins.sync_info:
                for w in ins.sync_info.on_wait:
                    if w.ant_name and w.ant_name.startswith("DMASW"):
                        dmasw = w
    assert prep is not None and dmasw is not None
    u = prep.sync_info.on_update[0]
    u.ant_name = dmasw.ant_name
    u.id = dmasw.id
    return nc


def kernel(x, rots, trans, intrins, post_rots, post_trans):
    from concourse.bass_utils import run_bass_kernel_spmd

    x = np.asarray(x, dtype=np.float32)
    flat, kept = _geometry_rows(rots, trans, intrins, post_rots, post_trans)
    ngc, pcap, gather, gslot, rows, sp_gather, sp_rows = _plan(flat, kept)
    nreg = ngc - 1
    groups = _split_groups(nreg)
    nscat = len(groups)

    xf = x.reshape(NP, CH).astype(BF)
    xf_ext = np.concatenate([xf, np.zeros((1, CH), BF)], axis=0)

    # identity scatter indices, [16, 8] wrap replicated across partitions
    t = np.arange(128)
    idx16 = np.zeros((16, 8), np.int16)
    idx16[t % 16, t // 16] = t.astype(np.int16)
    idxv = np.ascontiguousarray(np.tile(idx16, (8, 1)))

    in_maps = []
    for core in range(NCORES):
        gidx = gather[core * nreg:(core + 1) * nreg]        # [nreg,128,G]
        xdv = np.empty((ngc, 128, G * CH), BF)
        xdv[:nreg] = xf_ext[gidx].reshape(nreg, 128, G * CH)  # member-major
        # special chunk: one single-group voxel per partition, member-major
        xdv[nreg] = xf_ext[sp_gather[core]].reshape(128, G * CH)
        gs = np.ascontiguousarray(
            gslot[core * nreg:(core + 1) * nreg].T.astype(BF))  # [128,nreg]
        in_maps.append(dict(xd=xdv, gsd=gs, idxd=idxv))

    key = (ngc, pcap)
    if key not in _CACHE:
        _CACHE[key] = _build_kernel(ngc, pcap)
    nc = _CACHE[key]
    _CACHE["last_nc"] = nc

    res = run_bass_kernel_spmd(nc, in_maps, core_ids=list(range(NCORES)))

    # host-side scatter of compact voxel sums into the dense BEV grid
    nearly = sum(groups[:-1])
    kl_of_chunk = np.concatenate(
        [np.arange(sg) for sg in groups[:-1]])               # [nearly]
    g_of_chunk = np.repeat(np.arange(nscat - 1), groups[:-1])
    pooled = np.zeros((NROWS, CH), np.float32)
    for core in range(NCORES):
        o = np.asarray(res.results[core]["outc"], dtype=np.float32)
        ol = np.asarray(res.results[core]["outl"], dtype=np.float32)
        ol = ol[:, :(groups[-1] + 1) * CH]
        chunk_vals = np.empty((nreg, 128, CH), np.float32)
        chunk_vals[:nearly] = o[g_of_chunk, :, kl_of_chunk]
        lvals = ol.reshape(128, groups[-1] + 1, CH).transpose(1, 0, 2)
        chunk_vals[nearly:nreg] = lvals[:groups[-1]]
        r = rows[core * nreg:(core + 1) * nreg]              # [nreg,128]
        m = r >= 0
        pooled[r[m]] = chunk_vals[m]
        sp_vals = lvals[groups[-1]]                          # special column
        rs = sp_rows[core]
        ms = rs >= 0
        pooled[rs[ms]] = sp_vals[ms]
    final = pooled.reshape(B, NZ, NY, NX, CH) \
        .transpose(0, 1, 4, 2, 3).reshape(B, NZ * CH, NY, NX)
    return final
